# revision 1
# baseline (speedup 1.0000x reference)
"""Trainium2 Bass kernel for nn_Decoder (LSTM decoder + vocab projection).

Reference computation (B=64, S=64, E=256, H=512, V=32000):
    emb     = emb_table[target_seq]                      [B,S,E]
    lstm_in = concat([emb, ctx_broadcast], -1)           [B,S,E+H]
    pre     = lstm_in @ w_ih.T + b_ih + b_hh             [B,S,4H]
    per step: gates = pre_t + h @ w_hh.T ; LSTM update   [B,4H]
    logits  = concat([hs, ctx], -1) @ w_fc.T + b_fc      [B,S,V]

Sharding (8 cores):
  - batch 8-way for the embedding/pre/LSTM recurrence (8 batches/core)
  - vocab 8-way for the FC projection (w_fc shard resident in SBUF)
  - hidden states AllGathered across cores in 4 blocks of 16 steps
  - FC uses the split  logits = hs @ w_fc[:, :H].T + (ctx @ w_fc[:, H:].T
    + b_fc)  where the ctx part is computed ONCE per batch (not per step),
    halving FC FLOPs.

Everything on device is computed "transposed": states / gates keep the
feature dim on SBUF partitions, batch on the free dim.  Gate blocks are
host-permuted to [i, f, o, g] order so sigmoid/tanh each cover one
contiguous slice.

Token indexing: local token n = t*8 + b_local (t-major).  Global gathered
token tau = block*1024 + core*128 + (t%16)*8 + b_local.
"""

import numpy as np
import ml_dtypes

VOCAB, EMBED, HIDDEN = 32000, 256, 512
B, S = 64, 64
NCORES = 8
BL = B // NCORES          # 8 local batches
TOKL = S * BL             # 512 local tokens
TOK = S * B               # 4096 global tokens
G4 = 4 * HIDDEN           # 2048
KIN = EMBED + HIDDEN      # 768
KI = KIN // 128           # 6 k-tiles for pre
KH = HIDDEN // 128        # 4 k-tiles for recurrence / FC
GT = G4 // 128            # 16 gate tiles
VSH = 4096                # per-core (overlapping) vocab shard
VT = VSH // 128           # 32 vocab tiles
BLOCKS = [16, 16, 16, 8, 8]   # allgather block sizes (steps); small tail
NBLK = len(BLOCKS)
BSTART = [sum(BLOCKS[:j]) for j in range(NBLK)]      # first step of block
TAUB = [sum(BLOCKS[:j]) * B for j in range(NBLK)]    # tau base of block

V_STARTS = [0, 4000, 8000, 12000, 16000, 20000, 24000, VOCAB - VSH]

BF16 = ml_dtypes.bfloat16
FP8 = ml_dtypes.float8_e4m3

_CACHE = {}


def _build_program():
    import concourse.bass as bass
    import concourse.mybir as mybir
    import concourse.tile as tile
    from concourse import bacc

    bf = mybir.dt.bfloat16
    f8 = mybir.dt.float8e4
    f32 = mybir.dt.float32
    AF = mybir.ActivationFunctionType

    nc = bacc.Bacc(
        "TRN2",
        target_bir_lowering=False,
        debug=False,
        num_devices=NCORES,
    )

    # ---- DRAM I/O ----------------------------------------------------
    x_d = nc.dram_tensor("x_d", [KI, 128, TOKL], bf, kind="ExternalInput").ap()
    wih_d = nc.dram_tensor("wih_d", [KI, 128, G4], bf, kind="ExternalInput").ap()
    whh_d = nc.dram_tensor("whh_d", [KH, 128, G4], f8, kind="ExternalInput").ap()
    bias_d = nc.dram_tensor("bias_d", [128, GT], f32, kind="ExternalInput").ap()
    h0_d = nc.dram_tensor("h0_d", [128, KH, BL], bf, kind="ExternalInput").ap()
    c0_d = nc.dram_tensor("c0_d", [128, KH, BL], f32, kind="ExternalInput").ap()
    ctx_d = nc.dram_tensor("ctx_d", [KH, 128, B], bf, kind="ExternalInput").ap()
    wfh_d = nc.dram_tensor("wfh_d", [KH, 128, VSH], bf, kind="ExternalInput").ap()
    wfc_d = nc.dram_tensor("wfc_d", [KH, 128, VSH], bf, kind="ExternalInput").ap()
    bfc_d = nc.dram_tensor("bfc_d", [128, VT], f32, kind="ExternalInput").ap()
    log_d = nc.dram_tensor("log_d", [VSH, TOK], f32, kind="ExternalOutput").ap()

    with tile.TileContext(nc) as tc, \
         tc.tile_pool(name="singles", bufs=1) as sg, \
         tc.tile_pool(name="dramb", bufs=1, space="DRAM") as dramb:
        # ---- persistent SBUF tensors ---------------------------------
        x_sb = sg.tile([128, KI, TOKL], bf, name="x_sb", tag="x_sb")
        wih_sb = sg.tile([128, KI, G4], bf, name="wih_sb", tag="wih_sb")
        whh_sb = sg.tile([128, KH, G4], f8, name="whh_sb", tag="whh_sb")
        bias_sb = sg.tile([128, GT], f32, name="bias_sb", tag="bias_sb")
        h0_sb = sg.tile([128, KH, BL], bf, name="h0_sb", tag="h0_sb")
        c0_sb = sg.tile([128, KH, BL], f32, name="c0_sb", tag="c0_sb")
        ctx_sb = sg.tile([128, KH, B], bf, name="ctx_sb", tag="ctx_sb")
        wfh_sb = sg.tile([128, KH, VSH], bf, name="wfh_sb", tag="wfh_sb")
        wfc_sb = sg.tile([128, KH, VSH], bf, name="wfc_sb", tag="wfc_sb")
        bfc_sb = sg.tile([128, VT], f32, name="bfc_sb", tag="bfc_sb")
        pre_sb = sg.tile([128, GT, TOKL], bf, name="pre_sb", tag="pre_sb")
        ctxl_sb = sg.tile([128, VT, B], f32, name="ctxl_sb", tag="ctxl_sb")
        hs_sb = sg.tile([128, KH, TOK], bf, name="hs_sb", tag="hs_sb")
        stages = [
            sg.tile([128, KH, BLOCKS[j] * BL], bf, name=f"stage{j}",
                    tag=f"stage{j}")
            for j in range(NBLK)
        ]

        # ---- input DMAs ---------------------------------------------
        # spread input loads across queues so SP isn't a startup bottleneck
        nc.sync.dma_start(out=x_sb[:], in_=x_d.rearrange("k p n -> p k n"))
        nc.sync.dma_start(out=wih_sb[:], in_=wih_d.rearrange("k p n -> p k n"))
        nc.sync.dma_start(out=whh_sb[:], in_=whh_d.rearrange("k p n -> p k n"))
        nc.gpsimd.dma_start(out=bias_sb[:], in_=bias_d)
        nc.gpsimd.dma_start(out=h0_sb[:], in_=h0_d)
        nc.gpsimd.dma_start(out=c0_sb[:], in_=c0_d)
        nc.gpsimd.dma_start(out=ctx_sb[:], in_=ctx_d.rearrange("k p n -> p k n"))
        nc.gpsimd.dma_start(out=wfc_sb[:], in_=wfc_d.rearrange("k p n -> p k n"))
        nc.gpsimd.dma_start(out=wfh_sb[:], in_=wfh_d.rearrange("k p n -> p k n"))
        nc.gpsimd.dma_start(out=bfc_sb[:], in_=bfc_d)

        # ---- DRAM bounce buffers for the AllGathers ------------------
        ccin = []
        ccout = []
        for j in range(NBLK):
            bn = BLOCKS[j] * BL
            ci = dramb.tile([KH, 128, bn], bf, name=f"ccin{j}",
                            tag=f"ccin{j}")
            co = dramb.tile(
                [NCORES, KH, 128, bn], bf,
                addr_space="Shared", name=f"ccout{j}", tag=f"ccout{j}",
            )
            ccin.append(ci)
            ccout.append(co)

        with (
            tc.tile_pool(name="pmm", bufs=4, space="PSUM") as pmm,
            tc.tile_pool(name="pgate", bufs=2, space="PSUM") as pgate,
            tc.tile_pool(name="act", bufs=3) as actp,
            tc.tile_pool(name="cst", bufs=3) as cstp,
            tc.tile_pool(name="fout", bufs=4) as foutp,
        ):
            # ---- phase 1: pre = x @ w_ih.T + bias (transposed) -------
            for gt in range(GT):
                ps = pmm.tile([128, TOKL], f32, tag="mm512")
                for kt in range(KI):
                    nc.tensor.matmul(
                        ps[:],
                        lhsT=wih_sb[:, kt, gt * 128:(gt + 1) * 128],
                        rhs=x_sb[:, kt, :],
                        start=(kt == 0),
                        stop=(kt == KI - 1),
                    )
                nc.scalar.activation(
                    pre_sb[:, gt], ps[:], AF.Identity,
                    bias=bias_sb[:, gt:gt + 1],
                )

            # ---- phase 3: LSTM recurrence ----------------------------
            def blk_of(t):
                for j in range(NBLK):
                    if t < BSTART[j] + BLOCKS[j]:
                        return j, t - BSTART[j]
                raise AssertionError(t)

            c_prev = c0_sb
            for t in range(S):
                j, t16 = blk_of(t)
                if t == 0:
                    rhs_src = h0_sb
                    roff = 0
                else:
                    pj, pt = blk_of(t - 1)
                    rhs_src = stages[pj]
                    roff = pt * BL

                gp = pgate.tile([128, GT, BL], f32, tag="gates")
                for gt in range(GT):
                    for kt in range(KH):
                        nc.tensor.matmul(
                            gp[:, gt],
                            lhsT=whh_sb[:, kt, gt * 128:(gt + 1) * 128],
                            rhs=rhs_src[:, kt, roff:roff + BL],
                            start=(kt == 0),
                            stop=(kt == KH - 1),
                        )
                # add precomputed input contribution (in-place in PSUM)
                nc.vector.tensor_add(
                    gp[:], gp[:],
                    pre_sb[:, :, t * BL:(t + 1) * BL],
                )
                # activations: blocks are [i,f,o | g] after host permute
                sig = actp.tile([128, 3 * KH, BL], bf, tag="sig")
                gg = actp.tile([128, KH, BL], bf, tag="gg")
                nc.scalar.activation(sig[:], gp[:, 0:3 * KH], AF.Sigmoid)
                nc.scalar.activation(gg[:], gp[:, 3 * KH:GT], AF.Tanh)

                ig = cstp.tile([128, KH, BL], f32, tag="ig")
                fc = cstp.tile([128, KH, BL], f32, tag="fc")
                c_new = cstp.tile([128, KH, BL], f32, tag="c")
                tcn = cstp.tile([128, KH, BL], bf, tag="tc")
                nc.vector.tensor_mul(ig[:], sig[:, 0:KH], gg[:])
                nc.vector.tensor_mul(fc[:], sig[:, KH:2 * KH], c_prev[:])
                nc.vector.tensor_add(c_new[:], ig[:], fc[:])
                nc.scalar.activation(tcn[:], c_new[:], AF.Tanh)
                # write h in two halves so step t+1's k0/k1 matmuls can
                # start before the second half lands
                half = KH // 2
                st = stages[j][:, :, t16 * BL:(t16 + 1) * BL]
                nc.vector.tensor_mul(
                    st[:, 0:half], sig[:, 2 * KH:2 * KH + half],
                    tcn[:, 0:half],
                )
                nc.vector.tensor_mul(
                    st[:, half:KH], sig[:, 2 * KH + half:3 * KH],
                    tcn[:, half:KH],
                )
                c_prev = c_new

                if t16 == BLOCKS[j] - 1:
                    # ship this block: stage -> DRAM -> AllGather -> SBUF
                    bn = BLOCKS[j] * BL
                    nc.sync.dma_start(
                        out=ccin[j][:].rearrange("k p n -> p k n"),
                        in_=stages[j][:],
                    )
                    nc.gpsimd.collective_compute(
                        "AllGather",
                        mybir.AluOpType.bypass,
                        replica_groups=[list(range(NCORES))],
                        ins=[ccin[j][:]],
                        outs=[ccout[j][:]],
                    )
                    for c in range(NCORES):
                        nc.sync.dma_start(
                            out=hs_sb[:, :, TAUB[j] + c * bn:
                                      TAUB[j] + (c + 1) * bn],
                            in_=ccout[j][c].rearrange("k p n -> p k n"),
                        )

            # ---- phase 2 (emitted late = low priority; only needed by
            # the FC): ctx logits  ctx @ w_fc[:,H:].T + b_fc ----------
            for vt in range(VT):
                ps = pmm.tile([128, B], f32, tag="mm512")
                for kt in range(KH):
                    nc.tensor.matmul(
                        ps[:],
                        lhsT=wfc_sb[:, kt, vt * 128:(vt + 1) * 128],
                        rhs=ctx_sb[:, kt, :],
                        start=(kt == 0),
                        stop=(kt == KH - 1),
                    )
                nc.scalar.activation(
                    ctxl_sb[:, vt], ps[:], AF.Identity,
                    bias=bfc_sb[:, vt:vt + 1],
                )

            # ---- phase 4: FC  hs @ w_fc[:,:H].T + ctxl ---------------
            dma_engines = [nc.sync, nc.scalar, nc.gpsimd]
            ndma = 0
            for j in range(NBLK):
                W = BLOCKS[j] * B          # tau width of this block
                nch = W // 512             # 512-token chunks
                cpc = NCORES // nch        # cores covered per chunk
                for vt in range(VT):
                    fo = foutp.tile([128, 1024], f32, tag="fo")
                    for m in range(nch):
                        n0 = TAUB[j] + m * 512
                        ps = pmm.tile([128, 512], f32, tag="mm512")
                        for kt in range(KH):
                            nc.tensor.matmul(
                                ps[:],
                                lhsT=wfh_sb[:, kt, vt * 128:(vt + 1) * 128],
                                rhs=hs_sb[:, kt, n0:n0 + 512],
                                start=(kt == 0),
                                stop=(kt == KH - 1),
                            )
                        ctxv = (
                            ctxl_sb[:, vt, m * cpc * BL:(m + 1) * cpc * BL]
                            .rearrange("p (c b) -> p c b", b=BL)
                            .unsqueeze(2)
                            .broadcast_to([128, cpc, BLOCKS[j], BL])
                        )
                        nc.vector.tensor_add(
                            fo[:, m * 512:(m + 1) * 512]
                            .rearrange("p (c t b) -> p c t b",
                                       c=cpc, t=BLOCKS[j], b=BL),
                            ps[:].rearrange("p (c t b) -> p c t b",
                                            c=cpc, t=BLOCKS[j], b=BL),
                            ctxv,
                        )
                    # one store per (block, vtile); spread the dispatch
                    # cost across SP / ACT / Pool queues
                    eng = dma_engines[ndma % 3]
                    ndma += 1
                    eng.dma_start(
                        out=log_d[vt * 128:(vt + 1) * 128, TAUB[j]:TAUB[j] + W],
                        in_=fo[:, 0:W],
                    )

    nc.compile()
    return nc


def _get_nc():
    if "nc" not in _CACHE:
        _CACHE["nc"] = _build_program()
    return _CACHE["nc"]


def _block128(a):
    """[K, N] -> [K//128, 128, N] contiguous blocks."""
    k, n = a.shape
    return np.ascontiguousarray(a.reshape(k // 128, 128, n))


def _t_layout(a):
    """[BL, 512] state -> [128, KH, BL] transposed tile layout."""
    # out[p, kt, b] = a[b, kt*128 + p]
    return np.ascontiguousarray(a.T.reshape(KH, 128, BL).transpose(1, 0, 2))


def _prep_in_maps(target_seq, context, h, c, emb_table, w_ih, w_hh, b_ih,
                  b_hh, w_fc, b_fc):
    target_seq = np.asarray(target_seq)
    context = np.asarray(context, dtype=np.float32)
    h = np.asarray(h, dtype=np.float32)
    c = np.asarray(c, dtype=np.float32)
    emb_table = np.asarray(emb_table, dtype=np.float32)
    w_ih = np.asarray(w_ih, dtype=np.float32)
    w_hh = np.asarray(w_hh, dtype=np.float32)
    b_ih = np.asarray(b_ih, dtype=np.float32)
    b_hh = np.asarray(b_hh, dtype=np.float32)
    w_fc = np.asarray(w_fc, dtype=np.float32)
    b_fc = np.asarray(b_fc, dtype=np.float32)

    # gate-block permutation [i, f, g, o] -> [i, f, o, g]
    perm = np.concatenate([
        np.arange(0, HIDDEN),                    # i
        np.arange(HIDDEN, 2 * HIDDEN),           # f
        np.arange(3 * HIDDEN, 4 * HIDDEN),       # o
        np.arange(2 * HIDDEN, 3 * HIDDEN),       # g
    ])
    w_ih_p = w_ih[perm]
    w_hh_p = w_hh[perm]
    bias_p = (b_ih + b_hh)[perm]

    wih_d = _block128(w_ih_p.T.astype(BF16))          # [6,128,2048]
    whh_d = _block128(w_hh_p.T.astype(FP8))           # [4,128,2048]
    bias_d = np.ascontiguousarray(
        bias_p.reshape(GT, 128).T.astype(np.float32))  # [128,16]
    ctx_d = _block128(context.T.astype(BF16))          # [4,128,64]

    emb = emb_table[target_seq]                        # [B,S,E] f32

    in_maps = []
    for cid in range(NCORES):
        bs = slice(cid * BL, (cid + 1) * BL)
        # lstm_in transposed, local tokens n = t*8+b
        x_loc = np.concatenate(
            [
                emb[bs].transpose(1, 0, 2).reshape(TOKL, EMBED),
                np.tile(context[bs], (S, 1)),
            ],
            axis=1,
        )                                              # [512, 768]
        x_d = _block128(x_loc.T.astype(BF16))          # [6,128,512]
        vs = V_STARTS[cid]
        wfh_d = _block128(
            np.ascontiguousarray(w_fc[vs:vs + VSH, :HIDDEN].T).astype(BF16))
        wfc_d = _block128(
            np.ascontiguousarray(w_fc[vs:vs + VSH, HIDDEN:].T).astype(BF16))
        bfc_d = np.ascontiguousarray(
            b_fc[vs:vs + VSH].reshape(VT, 128).T.astype(np.float32))
        in_maps.append({
            "x_d": x_d,
            "wih_d": wih_d,
            "whh_d": whh_d,
            "bias_d": bias_d,
            "h0_d": _t_layout(h[bs]).astype(BF16),
            "c0_d": _t_layout(c[bs]).astype(np.float32),
            "ctx_d": ctx_d,
            "wfh_d": wfh_d,
            "wfc_d": wfc_d,
            "bfc_d": bfc_d,
        })
    return in_maps


def _assemble(results):
    """results: list of per-core {"log_d": [4096, 4096]} -> [B, S, V]."""
    full = np.empty((VOCAB, TOK), dtype=np.float32)
    for cid in range(NCORES):
        out_c = results[cid]["log_d"]                  # [4096, 4096]
        vs = V_STARTS[cid]
        r0 = cid * 4000 - vs
        full[cid * 4000:(cid + 1) * 4000] = out_c[r0:r0 + 4000]

    # tau = TAUB[j] + core*(BLOCKS[j]*8) + t_in*8 + b
    logits = np.empty((B, S, VOCAB), dtype=np.float32)
    for j in range(NBLK):
        W = BLOCKS[j] * B
        blk = full[:, TAUB[j]:TAUB[j] + W]             # [V, W]
        blk = blk.reshape(VOCAB, NCORES, BLOCKS[j], BL)
        # -> [core, b, t_in, V]
        logits[:, BSTART[j]:BSTART[j] + BLOCKS[j], :] = (
            blk.transpose(1, 3, 2, 0).reshape(B, BLOCKS[j], VOCAB)
        )
    return np.ascontiguousarray(logits)


def kernel(target_seq, context, h, c, emb_table, w_ih, w_hh, b_ih, b_hh,
           w_fc, b_fc):
    from concourse.bass_utils import run_bass_kernel_spmd

    in_maps = _prep_in_maps(target_seq, context, h, c, emb_table, w_ih,
                            w_hh, b_ih, b_hh, w_fc, b_fc)
    nc = _get_nc()
    res = run_bass_kernel_spmd(nc, in_maps, core_ids=list(range(NCORES)))
    return _assemble(res.results)



# revision 9
# speedup vs baseline: 1.2437x; 1.2437x over previous
"""Trainium2 Bass kernel for nn_Decoder (LSTM decoder + vocab projection).

Reference computation (B=64, S=64, E=256, H=512, V=32000):
    emb     = emb_table[target_seq]                      [B,S,E]
    lstm_in = concat([emb, ctx_broadcast], -1)           [B,S,E+H]
    per step: gates = lstm_in_t @ w_ih.T + b + h @ w_hh.T; LSTM update
    logits  = concat([hs, ctx], -1) @ w_fc.T + b_fc      [B,S,V]

Sharding (8 cores):
  - batch 8-way for the LSTM recurrence (8 batches/core)
  - vocab 8-way for the FC projection (w_fc shard resident in SBUF)
  - hidden states AllGathered (fp8) across cores in blocks of steps

Design notes:
  - gates accumulate x-part + h-part + bias directly in PSUM (bias rides
    as a 7th x k-tile whose input row is constant 1)
  - FC hs-part runs in fp8e4 with DoubleRow perf mode (2 k-tiles per
    matmul at 0.5 cycles/row)
  - ctx-part of FC: for DVE-converted chunks it is added during the
    PSUM->bf16 conversion (tensor_add with a broadcast ctxl); for
    ACT/Pool-converted chunks it is folded into PSUM by a small
    "selection matmul" (one-hot batch selector x ctxlT, bias row rides
    as a 65th selector row)
  - logits stored to DRAM as bf16 (upcast on host)
  - PSUM->SBUF conversion chunks round-robin across DVE / ACT / Pool

Token indexing: local token n = t*8 + b_local (t-major).  Global gathered
token tau = TAUB[j] + core*(BLOCKS[j]*8) + (t-BSTART[j])*8 + b_local.
"""

import numpy as np
import ml_dtypes

VOCAB, EMBED, HIDDEN = 32000, 256, 512
B, S = 64, 64
NCORES = 8
BL = B // NCORES          # 8 local batches
TOKL = S * BL             # 512 local tokens
TOK = S * B               # 4096 global tokens
G4 = 4 * HIDDEN           # 2048
KI = 7                    # 6 k-tiles of [emb|ctx] + 1 bias/ones k-tile
KH = HIDDEN // 128        # 4 k-tiles for recurrence / FC
GT = G4 // 128            # 16 gate tiles
VSH = 4096                # per-core (overlapping) vocab shard
VT = VSH // 128           # 32 vocab tiles
CH = 256                  # FC token chunk (DoubleRow moving limit)
BLOCKS = [16, 16, 16, 8, 4, 4]
NBLK = len(BLOCKS)
BSTART = [sum(BLOCKS[:j]) for j in range(NBLK)]      # first step of block
TAUB = [sum(BLOCKS[:j]) * B for j in range(NBLK)]    # tau base of block

V_STARTS = [0, 4000, 8000, 12000, 16000, 20000, 24000, VOCAB - VSH]

# selection-matrix variant per (block size, chunk index within block)
# 16-step: 4 chunks (vars 0-3), 8-step: 2 chunks (4-5), 4-step: 1 (6)
NSEL = 7

BF16 = ml_dtypes.bfloat16
FP8 = ml_dtypes.float8_e4m3

_CACHE = {}

# engine pattern for PSUM->bf16 conversion chunks (GPSIMD cannot read
# PSUM, so only ACT and DVE can drain FC accumulators)
CONV_PAT = ["A", "D"]


def _build_program():
    import concourse.bass as bass
    import concourse.mybir as mybir
    import concourse.tile as tile
    from concourse import bacc

    bf = mybir.dt.bfloat16
    f8 = mybir.dt.float8e4
    f32 = mybir.dt.float32
    AF = mybir.ActivationFunctionType
    DR = mybir.MatmulPerfMode.DoubleRow

    nc = bacc.Bacc(
        "TRN2",
        target_bir_lowering=False,
        debug=False,
        num_devices=NCORES,
    )

    # ---- DRAM I/O ----------------------------------------------------
    x_d = nc.dram_tensor("x_d", [KI, 128, TOKL], bf, kind="ExternalInput").ap()
    wih_d = nc.dram_tensor("wih_d", [KI, 128, G4], bf, kind="ExternalInput").ap()
    whh_d = nc.dram_tensor("whh_d", [KH, 128, G4], f8, kind="ExternalInput").ap()
    h0_d = nc.dram_tensor("h0_d", [128, KH, BL], bf, kind="ExternalInput").ap()
    c0_d = nc.dram_tensor("c0_d", [128, KH, BL], f32, kind="ExternalInput").ap()
    ctx_d = nc.dram_tensor("ctx_d", [KH, 128, B], bf, kind="ExternalInput").ap()
    wfh_d = nc.dram_tensor("wfh_d", [KH, 128, VSH], f8, kind="ExternalInput").ap()
    wfc_d = nc.dram_tensor("wfc_d", [KH, 128, VSH], bf, kind="ExternalInput").ap()
    bfc_d = nc.dram_tensor("bfc_d", [128, VT], f32, kind="ExternalInput").ap()
    bfcT_d = nc.dram_tensor("bfcT_d", [1, VT, 128], bf, kind="ExternalInput").ap()
    sel_d = nc.dram_tensor("sel_d", [65, NSEL, CH], f8, kind="ExternalInput").ap()
    log_d = nc.dram_tensor("log_d", [VSH, TOK], bf, kind="ExternalOutput").ap()

    with tile.TileContext(nc) as tc, \
         tc.tile_pool(name="singles", bufs=1) as sg, \
         tc.tile_pool(name="dramb", bufs=1, space="DRAM") as dramb:
        # ---- persistent SBUF tensors ---------------------------------
        x_sb = sg.tile([128, KI, TOKL], bf, name="x_sb", tag="x_sb")
        wih_sb = sg.tile([128, KI, G4], bf, name="wih_sb", tag="wih_sb")
        whh_sb = sg.tile([128, KH, G4], f8, name="whh_sb", tag="whh_sb")
        h0_sb = sg.tile([128, KH, BL], bf, name="h0_sb", tag="h0_sb")
        c0_sb = sg.tile([128, KH, BL], f32, name="c0_sb", tag="c0_sb")
        ctx_sb = sg.tile([128, KH, B], bf, name="ctx_sb", tag="ctx_sb")
        wfh_sb = sg.tile([128, KH, VSH], f8, name="wfh_sb", tag="wfh_sb")
        wfc_sb = sg.tile([128, KH, VSH], bf, name="wfc_sb", tag="wfc_sb")
        bfc_sb = sg.tile([128, VT], f32, name="bfc_sb", tag="bfc_sb")
        sel_sb = sg.tile([128, NSEL, CH], f8, name="sel_sb", tag="sel_sb")
        ctxl_sb = sg.tile([128, VT, B], f32, name="ctxl_sb", tag="ctxl_sb")
        ctxlT_sb = sg.tile([128, VT, 128], bf, name="ctxlT_sb", tag="ctxlT_sb")
        hs_sb = sg.tile([128, KH, TOK], f8, name="hs_sb", tag="hs_sb")
        stages = [
            sg.tile([128, KH, BLOCKS[j] * BL], f8, name=f"stage{j}",
                    tag=f"stage{j}")
            for j in range(NBLK)
        ]

        # ---- input DMAs (critical loads first) -----------------------
        nc.sync.dma_start(out=x_sb[:], in_=x_d.rearrange("k p n -> p k n"))
        nc.sync.dma_start(out=whh_sb[:], in_=whh_d.rearrange("k p n -> p k n"))
        nc.sync.dma_start(out=wih_sb[:], in_=wih_d.rearrange("k p n -> p k n"))
        nc.gpsimd.dma_start(out=h0_sb[:], in_=h0_d)
        nc.gpsimd.dma_start(out=c0_sb[:], in_=c0_d)
        nc.gpsimd.dma_start(out=ctx_sb[:], in_=ctx_d.rearrange("k p n -> p k n"))
        nc.gpsimd.dma_start(out=bfc_sb[:], in_=bfc_d)
        nc.gpsimd.dma_start(out=sel_sb[0:65], in_=sel_d)
        # lower-priority weight loads (needed from first FC block on)
        nc.scalar.dma_start(out=wfh_sb[:], in_=wfh_d.rearrange("k p n -> p k n"))
        nc.scalar.dma_start(out=wfc_sb[:], in_=wfc_d.rearrange("k p n -> p k n"))
        nc.scalar.dma_start(out=ctxlT_sb[64:65], in_=bfcT_d)

        # ---- DRAM bounce buffers for the AllGathers ------------------
        ccin = []
        ccout = []
        for j in range(NBLK):
            bn = BLOCKS[j] * BL
            ci = dramb.tile([KH, 128, bn], f8, name=f"ccin{j}",
                            tag=f"ccin{j}")
            co = dramb.tile(
                [NCORES, KH, 128, bn], f8,
                addr_space="Shared", name=f"ccout{j}", tag=f"ccout{j}",
            )
            ccin.append(ci)
            ccout.append(co)

        with (
            tc.tile_pool(name="pgate", bufs=2, space="PSUM") as pgate,
            tc.tile_pool(name="pfc", bufs=6, space="PSUM") as pfc,
            tc.tile_pool(name="act", bufs=3) as actp,
            tc.tile_pool(name="cst", bufs=3) as cstp,
            tc.tile_pool(name="fout", bufs=10) as foutp,
        ):
            # gate tile emission order: i(0-3), f(4-7), g(12-15), o(8-11)
            GORDER = [0, 1, 2, 3, 4, 5, 6, 7, 12, 13, 14, 15, 8, 9, 10, 11]

            def blk_of(t):
                for j in range(NBLK):
                    if t < BSTART[j] + BLOCKS[j]:
                        return j, t - BSTART[j]
                raise AssertionError(t)

            # ---- LSTM recurrence ------------------------------------
            c_prev = c0_sb
            h_prev = h0_sb
            for t in range(S):
                j, t16 = blk_of(t)

                gp = pgate.tile([128, GT, BL], f32, tag="gates")
                # one contiguous accumulation group per gate tile:
                # x-part (+bias k-tile) then h-part (PSUM zero regions are
                # bank-granular, so groups must not interleave)
                for gt in GORDER:
                    for ki in range(KI):
                        nc.tensor.matmul(
                            gp[:, gt],
                            lhsT=wih_sb[:, ki, gt * 128:(gt + 1) * 128],
                            rhs=x_sb[:, ki, t * BL:(t + 1) * BL],
                            start=(ki == 0),
                            stop=False,
                        )
                    for kt in range(KH):
                        nc.tensor.matmul(
                            gp[:, gt],
                            lhsT=whh_sb[:, kt, gt * 128:(gt + 1) * 128],
                            rhs=h_prev[:, kt, :],
                            start=False,
                            stop=(kt == KH - 1),
                        )

                # activations: gate blocks are [i, f, o, g] (host permute)
                sif = actp.tile([128, 8, BL], bf, tag="sif")
                gg = actp.tile([128, KH, BL], bf, tag="gg")
                so = actp.tile([128, KH, BL], bf, tag="so")
                nc.scalar.activation(sif[:], gp[:, 0:8], AF.Sigmoid)
                nc.scalar.activation(gg[:], gp[:, 12:16], AF.Tanh)
                nc.scalar.activation(so[:], gp[:, 8:12], AF.Sigmoid)

                ig = cstp.tile([128, KH, BL], f32, tag="ig")
                fc = cstp.tile([128, KH, BL], f32, tag="fc")
                c_new = cstp.tile([128, KH, BL], f32, tag="c")
                tcn = cstp.tile([128, KH, BL], bf, tag="tc")
                h_new = cstp.tile([128, KH, BL], bf, tag="h")
                nc.vector.tensor_mul(ig[:], sif[:, 0:KH], gg[:])
                nc.gpsimd.tensor_mul(fc[:], sif[:, KH:8], c_prev[:])
                nc.vector.tensor_add(c_new[:], ig[:], fc[:])
                nc.scalar.activation(tcn[:], c_new[:], AF.Tanh)
                # write h in two halves so step t+1's k0/k1 matmuls can
                # start before the second half lands
                half = KH // 2
                nc.vector.tensor_mul(
                    h_new[:, 0:half], so[:, 0:half], tcn[:, 0:half])
                nc.vector.tensor_mul(
                    h_new[:, half:KH], so[:, half:KH], tcn[:, half:KH])
                # fp8 copy for the gather (off critical path, on Pool)
                st = stages[j][:, :, t16 * BL:(t16 + 1) * BL]
                nc.gpsimd.tensor_mul(st, so[:], tcn[:])
                c_prev = c_new
                h_prev = h_new

                if t16 == BLOCKS[j] - 1:
                    # ship this block: stage -> DRAM -> AllGather -> SBUF
                    bn = BLOCKS[j] * BL
                    nc.sync.dma_start(
                        out=ccin[j][:].rearrange("k p n -> p k n"),
                        in_=stages[j][:],
                    )
                    nc.gpsimd.collective_compute(
                        "AllGather",
                        mybir.AluOpType.bypass,
                        replica_groups=[list(range(NCORES))],
                        ins=[ccin[j][:]],
                        outs=[ccout[j][:]],
                    )
                    for c in range(NCORES):
                        nc.sync.dma_start(
                            out=hs_sb[:, :, TAUB[j] + c * bn:
                                      TAUB[j] + (c + 1) * bn],
                            in_=ccout[j][c].rearrange("k p n -> p k n"),
                        )

            # ---- ctx-part of FC (low priority; needed by first FC) ---
            # ctxl[v, b] for DVE-path adds
            for vt in range(VT):
                ps = pfc.tile([128, CH], f32, tag="pfc")
                for kt in range(KH):
                    nc.tensor.matmul(
                        ps[:, 0:B],
                        lhsT=wfc_sb[:, kt, vt * 128:(vt + 1) * 128],
                        rhs=ctx_sb[:, kt, :],
                        start=(kt == 0),
                        stop=(kt == KH - 1),
                    )
                nc.scalar.activation(
                    ctxl_sb[:, vt], ps[:, 0:B], AF.Identity,
                    bias=bfc_sb[:, vt:vt + 1],
                )
            # ctxlT[b, v] (bf16) for the selection matmuls
            for vt in range(VT):
                ps = pfc.tile([128, CH], f32, tag="pfc")
                for kt in range(KH):
                    nc.tensor.matmul(
                        ps[0:B, 0:128],
                        lhsT=ctx_sb[:, kt, :],
                        rhs=wfc_sb[:, kt, vt * 128:(vt + 1) * 128],
                        start=(kt == 0),
                        stop=(kt == KH - 1),
                    )
                nc.scalar.activation(ctxlT_sb[0:B, vt], ps[0:B, 0:128], AF.Copy)

            # ---- FC: logits = hs @ wfh.T (+ctx +bias), vocab-sharded -
            selv_of = {16: [0, 1, 2, 3], 8: [4, 5], 4: [6]}
            nconv = 0
            for j in range(NBLK):
                W = BLOCKS[j] * B          # tau width of this block
                hw_ = min(W, 512)          # store granularity (tokens)
                nch = hw_ // CH            # conversion chunks per store
                for hb in range(W // hw_):
                    for q in range(8):     # vt quads
                        fo = foutp.tile([128, 4, 512], bf, tag="fo")
                        for i in range(4):
                            vt = q * 4 + i
                            for cm in range(nch):
                                m = hb * nch + cm
                                n0 = TAUB[j] + m * CH
                                eng = CONV_PAT[nconv % len(CONV_PAT)]
                                nconv += 1
                                ps = pfc.tile([128, CH], f32, tag="pfc")
                                for kp in range(2):
                                    nc.tensor.matmul(
                                        ps[:],
                                        lhsT=wfh_sb[:, 2 * kp:2 * kp + 2,
                                                    vt * 128:(vt + 1) * 128],
                                        rhs=hs_sb[:, 2 * kp:2 * kp + 2,
                                                  n0:n0 + CH],
                                        start=(kp == 0),
                                        stop=(eng == "D" and kp == 1),
                                        perf_mode=DR,
                                    )
                                out_sl = fo[:, i, cm * CH:(cm + 1) * CH]
                                if eng == "D":
                                    # ctx+bias via broadcast add on DVE
                                    cpc = NCORES * CH // W
                                    ctxv = (
                                        ctxl_sb[:, vt,
                                                m * cpc * BL:(m + 1) * cpc * BL]
                                        .rearrange("p (c b) -> p c b", b=BL)
                                        .unsqueeze(2)
                                        .broadcast_to(
                                            [128, cpc, BLOCKS[j], BL])
                                    )
                                    nc.vector.tensor_add(
                                        out_sl.rearrange(
                                            "p (c t b) -> p c t b",
                                            c=cpc, t=BLOCKS[j], b=BL),
                                        ps[:].rearrange(
                                            "p (c t b) -> p c t b",
                                            c=cpc, t=BLOCKS[j], b=BL),
                                        ctxv,
                                    )
                                else:
                                    # ctx+bias via selection matmul in PSUM
                                    sv = selv_of[BLOCKS[j]][m]
                                    nc.tensor.matmul(
                                        ps[:],
                                        lhsT=ctxlT_sb[0:65, vt, :],
                                        rhs=sel_sb[0:65, sv, :],
                                        start=False,
                                        stop=True,
                                    )
                                    nc.scalar.activation(
                                        out_sl, ps[:], AF.Copy)
                        nc.sync.dma_start(
                            out=log_d[q * 512:(q + 1) * 512,
                                      TAUB[j] + hb * hw_:
                                      TAUB[j] + hb * hw_ + hw_]
                            .rearrange("(i p) n -> p i n", p=128),
                            in_=fo[:, :, 0:hw_],
                        )

    nc.compile()
    return nc


def _get_nc():
    if "nc" not in _CACHE:
        _CACHE["nc"] = _build_program()
    return _CACHE["nc"]


def _block128(a):
    """[K, N] -> [K//128, 128, N] contiguous blocks."""
    k, n = a.shape
    return np.ascontiguousarray(a.reshape(k // 128, 128, n))


def _t_layout(a):
    """[BL, 512] state -> [128, KH, BL] transposed tile layout."""
    return np.ascontiguousarray(a.T.reshape(KH, 128, BL).transpose(1, 0, 2))


def _build_sel():
    """Selection matrices [65, NSEL, 256] fp8.

    Variant layout (chunk of 256 gathered tokens, pos = (c_rel, t, b)):
      16-step blocks, chunk m in 0..3: batch = 16m + 8*(pos>>7) + (pos&7)
      8-step blocks, chunk m in 0..1:  batch = 32m + 8*(pos>>6) + (pos&7)
      4-step blocks, single chunk:     batch = 8*(pos>>5) + (pos&7)
    Row 64 is the all-ones bias row.
    """
    sel = np.zeros((65, NSEL, CH), np.float32)
    pos = np.arange(CH)
    for m in range(4):
        bt = 16 * m + 8 * (pos >> 7) + (pos & 7)
        sel[bt, m, pos] = 1.0
    for m in range(2):
        bt = 32 * m + 8 * (pos >> 6) + (pos & 7)
        sel[bt, 4 + m, pos] = 1.0
    bt = 8 * (pos >> 5) + (pos & 7)
    sel[bt, 6, pos] = 1.0
    sel[64, :, :] = 1.0
    return sel.astype(FP8)


def _prep_in_maps(target_seq, context, h, c, emb_table, w_ih, w_hh, b_ih,
                  b_hh, w_fc, b_fc):
    target_seq = np.asarray(target_seq)
    context = np.asarray(context, dtype=np.float32)
    h = np.asarray(h, dtype=np.float32)
    c = np.asarray(c, dtype=np.float32)
    emb_table = np.asarray(emb_table, dtype=np.float32)
    w_ih = np.asarray(w_ih, dtype=np.float32)
    w_hh = np.asarray(w_hh, dtype=np.float32)
    b_ih = np.asarray(b_ih, dtype=np.float32)
    b_hh = np.asarray(b_hh, dtype=np.float32)
    w_fc = np.asarray(w_fc, dtype=np.float32)
    b_fc = np.asarray(b_fc, dtype=np.float32)

    # gate-block permutation [i, f, g, o] -> [i, f, o, g]
    perm = np.concatenate([
        np.arange(0, HIDDEN),                    # i
        np.arange(HIDDEN, 2 * HIDDEN),           # f
        np.arange(3 * HIDDEN, 4 * HIDDEN),       # o
        np.arange(2 * HIDDEN, 3 * HIDDEN),       # g
    ])
    w_ih_p = w_ih[perm]
    w_hh_p = w_hh[perm]
    bias_p = (b_ih + b_hh)[perm]

    # w_ih with the bias as a 7th k-tile (input row is constant 1)
    wih_full = np.zeros((KI * 128, G4), np.float32)
    wih_full[:768] = w_ih_p.T
    wih_full[768] = bias_p
    wih_d = _block128(wih_full.astype(BF16))           # [7,128,2048]
    whh_d = _block128(w_hh_p.T.astype(FP8))            # [4,128,2048]
    ctx_d = _block128(context.T.astype(BF16))          # [4,128,64]
    sel_dm = np.ascontiguousarray(
        _build_sel())                                  # [65,NSEL,256]
    bfcT_d = np.ascontiguousarray(
        b_fc[:VSH].reshape(1, VT, 128)).astype(BF16)   # placeholder per-core

    emb = emb_table[target_seq]                        # [B,S,E] f32

    in_maps = []
    for cid in range(NCORES):
        bs = slice(cid * BL, (cid + 1) * BL)
        # lstm_in transposed, local tokens n = t*8+b; 897th row = 1
        x_loc = np.zeros((TOKL, KI * 128), np.float32)
        x_loc[:, :EMBED] = emb[bs].transpose(1, 0, 2).reshape(TOKL, EMBED)
        x_loc[:, EMBED:768] = np.tile(context[bs], (S, 1))
        x_loc[:, 768] = 1.0
        x_d = _block128(x_loc.T.astype(BF16))          # [7,128,512]
        vs = V_STARTS[cid]
        wfh_d = _block128(
            np.ascontiguousarray(w_fc[vs:vs + VSH, :HIDDEN].T).astype(FP8))
        wfc_d = _block128(
            np.ascontiguousarray(w_fc[vs:vs + VSH, HIDDEN:].T).astype(BF16))
        bfc_d = np.ascontiguousarray(
            b_fc[vs:vs + VSH].reshape(VT, 128).T.astype(np.float32))
        bfcT_c = np.ascontiguousarray(
            b_fc[vs:vs + VSH].reshape(1, VT, 128)).astype(BF16)
        in_maps.append({
            "x_d": x_d,
            "wih_d": wih_d,
            "whh_d": whh_d,
            "h0_d": _t_layout(h[bs]).astype(BF16),
            "c0_d": _t_layout(c[bs]).astype(np.float32),
            "ctx_d": ctx_d,
            "wfh_d": wfh_d,
            "wfc_d": wfc_d,
            "bfc_d": bfc_d,
            "bfcT_d": bfcT_c,
            "sel_d": sel_dm,
        })
    return in_maps


def _assemble(results):
    """results: list of per-core {"log_d": [4096, 4096] bf16} -> [B, S, V]."""
    full = np.empty((VOCAB, TOK), dtype=np.float32)
    for cid in range(NCORES):
        out_c = np.asarray(results[cid]["log_d"], dtype=np.float32)
        vs = V_STARTS[cid]
        r0 = cid * 4000 - vs
        full[cid * 4000:(cid + 1) * 4000] = out_c[r0:r0 + 4000]

    # tau = TAUB[j] + core*(BLOCKS[j]*8) + t_in*8 + b
    logits = np.empty((B, S, VOCAB), dtype=np.float32)
    for j in range(NBLK):
        W = BLOCKS[j] * B
        blk = full[:, TAUB[j]:TAUB[j] + W]             # [V, W]
        blk = blk.reshape(VOCAB, NCORES, BLOCKS[j], BL)
        logits[:, BSTART[j]:BSTART[j] + BLOCKS[j], :] = (
            blk.transpose(1, 3, 2, 0).reshape(B, BLOCKS[j], VOCAB)
        )
    return np.ascontiguousarray(logits)


def kernel(target_seq, context, h, c, emb_table, w_ih, w_hh, b_ih, b_hh,
           w_fc, b_fc):
    from concourse.bass_utils import run_bass_kernel_spmd

    in_maps = _prep_in_maps(target_seq, context, h, c, emb_table, w_ih,
                            w_hh, b_ih, b_hh, w_fc, b_fc)
    nc = _get_nc()
    res = run_bass_kernel_spmd(nc, in_maps, core_ids=list(range(NCORES)))
    return _assemble(res.results)


# revision 15
# speedup vs baseline: 1.2594x; 1.0126x over previous
"""Trainium2 Bass kernel for nn_Decoder (LSTM decoder + vocab projection).

Reference computation (B=64, S=64, E=256, H=512, V=32000):
    emb     = emb_table[target_seq]                      [B,S,E]
    lstm_in = concat([emb, ctx_broadcast], -1)           [B,S,E+H]
    per step: gates = lstm_in_t @ w_ih.T + b + h @ w_hh.T; LSTM update
    logits  = concat([hs, ctx], -1) @ w_fc.T + b_fc      [B,S,V]

Sharding (8 cores):
  - batch 8-way for the LSTM recurrence (8 batches/core)
  - vocab 8-way for the FC projection (w_fc shard resident in SBUF)
  - hidden states AllGathered (fp8) across cores in blocks of steps

Design notes:
  - gates accumulate x-part + h-part + bias directly in PSUM (bias rides
    as a 7th x k-tile whose input row is constant 1)
  - FC hs-part runs in fp8e4 with DoubleRow perf mode (2 k-tiles per
    matmul at 0.5 cycles/row)
  - ctx-part of FC: for DVE-converted chunks it is added during the
    PSUM->bf16 conversion (tensor_add with a broadcast ctxl); for
    ACT/Pool-converted chunks it is folded into PSUM by a small
    "selection matmul" (one-hot batch selector x ctxlT, bias row rides
    as a 65th selector row)
  - logits stored to DRAM as bf16 (upcast on host)
  - PSUM->SBUF conversion chunks round-robin across DVE / ACT / Pool

Token indexing: local token n = t*8 + b_local (t-major).  Global gathered
token tau = TAUB[j] + core*(BLOCKS[j]*8) + (t-BSTART[j])*8 + b_local.
"""

import numpy as np
import ml_dtypes

VOCAB, EMBED, HIDDEN = 32000, 256, 512
B, S = 64, 64
NCORES = 8
BL = B // NCORES          # 8 local batches
TOKL = S * BL             # 512 local tokens
TOK = S * B               # 4096 global tokens
G4 = 4 * HIDDEN           # 2048
KI = 7                    # 6 k-tiles of [emb|ctx] + 1 bias/ones k-tile
KH = HIDDEN // 128        # 4 k-tiles for recurrence / FC
GT = G4 // 128            # 16 gate tiles
VSH = 4096                # per-core (overlapping) vocab shard
VT = VSH // 128           # 32 vocab tiles
CH = 256                  # FC DoubleRow moving-dim limit (tokens)
BLOCKS = [16, 16, 16, 8, 8]
NBLK = len(BLOCKS)
BSTART = [sum(BLOCKS[:j]) for j in range(NBLK)]      # first step of block
TAUB = [sum(BLOCKS[:j]) * B for j in range(NBLK)]    # tau base of block

V_STARTS = [0, 4000, 8000, 12000, 16000, 20000, 24000, VOCAB - VSH]

# selection-matrix variant per (block size, chunk index within block)
# 16-step: 4 chunks (vars 0-3), 8-step: 2 chunks (4-5), 4-step: 1 (6)
NSEL = 7

BF16 = ml_dtypes.bfloat16
FP8 = ml_dtypes.float8_e4m3

_CACHE = {}

# engine pattern for PSUM->bf16 conversion (GPSIMD cannot read PSUM, so
# only ACT and DVE can drain FC accumulators; DVE gets the larger share
# because ACT also runs the per-step gate activations)
CONV_PAT = ["A", "D", "D", "A", "D", "D", "A", "D", "D", "A", "D"]

# per-block scheduling gates: the recurrence step whose completion
# approximates when block j's AllGather lands (keeps the static PE/SP
# schedule from hoisting FC ahead of the real collective latency)
STEP_NS = 2100.0
STARTUP_NS = 19000.0


def _gate_steps():
    gates = []
    dev_free = 0.0
    for j in range(NBLK):
        t_end = STARTUP_NS + STEP_NS * (BSTART[j] + BLOCKS[j])
        ready = t_end + 1600.0
        out_bytes = NCORES * HIDDEN * BLOCKS[j] * BL  # fp8
        dur = 15000.0 + out_bytes / 40.0
        dev_free = max(ready, dev_free) + dur
        g = int((dev_free - STARTUP_NS) / STEP_NS)
        gates.append(min(max(g, BSTART[j] + BLOCKS[j]), S - 1))
    return gates


def _build_program():
    import concourse.bass as bass
    import concourse.mybir as mybir
    import concourse.tile as tile
    from concourse import bacc

    bf = mybir.dt.bfloat16
    f8 = mybir.dt.float8e4
    f32 = mybir.dt.float32
    AF = mybir.ActivationFunctionType
    DR = mybir.MatmulPerfMode.DoubleRow

    nc = bacc.Bacc(
        "TRN2",
        target_bir_lowering=False,
        debug=False,
        num_devices=NCORES,
    )

    # ---- DRAM I/O ----------------------------------------------------
    x_d = nc.dram_tensor("x_d", [KI, 128, TOKL], bf, kind="ExternalInput").ap()
    wih_d = nc.dram_tensor("wih_d", [KI, 128, G4], bf, kind="ExternalInput").ap()
    whh_d = nc.dram_tensor("whh_d", [KH, 128, G4], f8, kind="ExternalInput").ap()
    h0_d = nc.dram_tensor("h0_d", [128, KH, BL], bf, kind="ExternalInput").ap()
    c0_d = nc.dram_tensor("c0_d", [128, KH, BL], f32, kind="ExternalInput").ap()
    ctx_d = nc.dram_tensor("ctx_d", [KH, 128, B], bf, kind="ExternalInput").ap()
    wfh_d = nc.dram_tensor("wfh_d", [KH, 128, VSH], f8, kind="ExternalInput").ap()
    wfc_d = nc.dram_tensor("wfc_d", [KH, 128, VSH], bf, kind="ExternalInput").ap()
    bfc_d = nc.dram_tensor("bfc_d", [128, VT], f32, kind="ExternalInput").ap()
    bfcT_d = nc.dram_tensor("bfcT_d", [1, VT, 128], bf, kind="ExternalInput").ap()
    sel_d = nc.dram_tensor("sel_d", [65, NSEL, CH], f8, kind="ExternalInput").ap()
    log_d = nc.dram_tensor("log_d", [VSH, TOK], bf, kind="ExternalOutput").ap()

    with tile.TileContext(nc) as tc, \
         tc.tile_pool(name="singles", bufs=1) as sg, \
         tc.tile_pool(name="dramb", bufs=1, space="DRAM") as dramb:
        # ---- persistent SBUF tensors ---------------------------------
        x_sb = sg.tile([128, KI, TOKL], bf, name="x_sb", tag="x_sb")
        wih_sb = sg.tile([128, KI, G4], bf, name="wih_sb", tag="wih_sb")
        whh_sb = sg.tile([128, KH, G4], f8, name="whh_sb", tag="whh_sb")
        h0_sb = sg.tile([128, KH, BL], bf, name="h0_sb", tag="h0_sb")
        c0_sb = sg.tile([128, KH, BL], f32, name="c0_sb", tag="c0_sb")
        ctx_sb = sg.tile([128, KH, B], bf, name="ctx_sb", tag="ctx_sb")
        wfh_sb = sg.tile([128, KH, VSH], f8, name="wfh_sb", tag="wfh_sb")
        wfc_sb = sg.tile([128, KH, VSH], bf, name="wfc_sb", tag="wfc_sb")
        bfc_sb = sg.tile([128, VT], f32, name="bfc_sb", tag="bfc_sb")
        sel_sb = sg.tile([128, NSEL, CH], f8, name="sel_sb", tag="sel_sb")
        ctxl_sb = sg.tile([128, VT, B], f32, name="ctxl_sb", tag="ctxl_sb")
        ctxlT_sb = sg.tile([128, VT, 128], bf, name="ctxlT_sb", tag="ctxlT_sb")
        hs_sb = sg.tile([128, KH, TOK], f8, name="hs_sb", tag="hs_sb")
        stages = [
            sg.tile([128, KH, BLOCKS[j] * BL], f8, name=f"stage{j}",
                    tag=f"stage{j}")
            for j in range(NBLK)
        ]

        # ---- input DMAs (critical loads first) -----------------------
        nc.sync.dma_start(out=x_sb[:], in_=x_d.rearrange("k p n -> p k n"))
        nc.sync.dma_start(out=whh_sb[:], in_=whh_d.rearrange("k p n -> p k n"))
        nc.sync.dma_start(out=wih_sb[:], in_=wih_d.rearrange("k p n -> p k n"))
        nc.gpsimd.dma_start(out=h0_sb[:], in_=h0_d)
        nc.gpsimd.dma_start(out=c0_sb[:], in_=c0_d)
        nc.gpsimd.dma_start(out=ctx_sb[:], in_=ctx_d.rearrange("k p n -> p k n"))
        nc.gpsimd.dma_start(out=bfc_sb[:], in_=bfc_d)
        nc.gpsimd.dma_start(out=sel_sb[0:65], in_=sel_d)
        # lower-priority weight loads (needed from first FC block on);
        # keep them off ACT/DVE so they never block the cell ops
        nc.sync.dma_start(out=wfh_sb[:], in_=wfh_d.rearrange("k p n -> p k n"))
        nc.sync.dma_start(out=wfc_sb[:], in_=wfc_d.rearrange("k p n -> p k n"))
        nc.gpsimd.dma_start(out=ctxlT_sb[64:65], in_=bfcT_d)

        # ---- DRAM bounce buffers for the AllGathers ------------------
        ccin = []
        ccout = []
        for j in range(NBLK):
            bn = BLOCKS[j] * BL
            ci = dramb.tile([KH, 128, bn], f8, name=f"ccin{j}",
                            tag=f"ccin{j}")
            co = dramb.tile(
                [NCORES, KH, 128, bn], f8,
                addr_space="Shared", name=f"ccout{j}", tag=f"ccout{j}",
            )
            ccin.append(ci)
            ccout.append(co)

        with (
            tc.tile_pool(name="pgate", bufs=2, space="PSUM") as pgate,
            tc.tile_pool(name="pfc", bufs=6, space="PSUM") as pfc,
            tc.tile_pool(name="act", bufs=3) as actp,
            tc.tile_pool(name="cst", bufs=3) as cstp,
            tc.tile_pool(name="fout", bufs=10) as foutp,
        ):
            # gate tile emission order: i(0-3), f(4-7), g(12-15), o(8-11)
            GORDER = [0, 1, 2, 3, 4, 5, 6, 7, 12, 13, 14, 15, 8, 9, 10, 11]

            def blk_of(t):
                for j in range(NBLK):
                    if t < BSTART[j] + BLOCKS[j]:
                        return j, t - BSTART[j]
                raise AssertionError(t)

            # ---- LSTM recurrence ------------------------------------
            c_prev = c0_sb
            h_prev = h0_sb
            hwr_ops = []            # per-step gate instruction (h write)
            for t in range(S):
                j, t16 = blk_of(t)

                gp = pgate.tile([128, GT, BL], f32, tag="gates")
                # one contiguous accumulation group per gate tile:
                # x-part (+bias k-tile) then h-part (PSUM zero regions are
                # bank-granular, so groups must not interleave)
                for gt in GORDER:
                    for ki in range(KI):
                        nc.tensor.matmul(
                            gp[:, gt],
                            lhsT=wih_sb[:, ki, gt * 128:(gt + 1) * 128],
                            rhs=x_sb[:, ki, t * BL:(t + 1) * BL],
                            start=(ki == 0),
                            stop=False,
                        )
                    for kt in range(KH):
                        nc.tensor.matmul(
                            gp[:, gt],
                            lhsT=whh_sb[:, kt, gt * 128:(gt + 1) * 128],
                            rhs=h_prev[:, kt, :],
                            start=False,
                            stop=(kt == KH - 1),
                        )

                # activations: gate blocks are [i, f, o, g] (host permute)
                sif = actp.tile([128, 8, BL], bf, tag="sif")
                gg = actp.tile([128, KH, BL], bf, tag="gg")
                so = actp.tile([128, KH, BL], bf, tag="so")
                nc.scalar.activation(sif[:], gp[:, 0:8], AF.Sigmoid)
                nc.scalar.activation(gg[:], gp[:, 12:16], AF.Tanh)
                nc.scalar.activation(so[:], gp[:, 8:12], AF.Sigmoid)

                ig = cstp.tile([128, KH, BL], f32, tag="ig")
                fc = cstp.tile([128, KH, BL], f32, tag="fc")
                c_new = cstp.tile([128, KH, BL], f32, tag="c")
                tcn = cstp.tile([128, KH, BL], bf, tag="tc")
                h_new = cstp.tile([128, KH, BL], bf, tag="h")
                nc.vector.tensor_mul(ig[:], sif[:, 0:KH], gg[:])
                nc.gpsimd.tensor_mul(fc[:], sif[:, KH:8], c_prev[:])
                nc.vector.tensor_add(c_new[:], ig[:], fc[:])
                nc.scalar.activation(tcn[:], c_new[:], AF.Tanh)
                # write h in two halves so step t+1's k0/k1 matmuls can
                # start before the second half lands
                half = KH // 2
                nc.vector.tensor_mul(
                    h_new[:, 0:half], so[:, 0:half], tcn[:, 0:half])
                hw_op = nc.vector.tensor_mul(
                    h_new[:, half:KH], so[:, half:KH], tcn[:, half:KH])
                hwr_ops.append(hw_op)
                # fp8 copy for the gather (off critical path, on Pool)
                st = stages[j][:, :, t16 * BL:(t16 + 1) * BL]
                nc.gpsimd.tensor_mul(st, so[:], tcn[:])
                c_prev = c_new
                h_prev = h_new

                if t16 == BLOCKS[j] - 1:
                    # ship this block: stage -> DRAM -> AllGather
                    nc.sync.dma_start(
                        out=ccin[j][:].rearrange("k p n -> p k n"),
                        in_=stages[j][:],
                    )
                    nc.gpsimd.collective_compute(
                        "AllGather",
                        mybir.AluOpType.bypass,
                        replica_groups=[list(range(NCORES))],
                        ins=[ccin[j][:]],
                        outs=[ccout[j][:]],
                    )

            # ---- gather landings: gated so the static schedule matches
            # the real collective latency (the scheduling pass models
            # collectives as instant and would hoist FC into the
            # recurrence, head-of-line blocking the PE queue) ----------
            from concourse.tile_rust import add_dep_helper
            gates = _gate_steps()
            for j in range(NBLK):
                bn = BLOCKS[j] * BL
                for c in range(NCORES):
                    ld = nc.sync.dma_start(
                        out=hs_sb[:, :, TAUB[j] + c * bn:
                                  TAUB[j] + (c + 1) * bn],
                        in_=ccout[j][c].rearrange("k p n -> p k n"),
                    )
                    add_dep_helper(
                        hwr_ops[gates[j]].ins, ld.ins,
                        sync=True, reason=f"gate hs load blk{j}",
                    )

            # ---- ctx-part of FC (low priority; needed by first FC) ---
            # ctxl[v, b] for DVE-path adds
            for vt in range(VT):
                ps = pfc.tile([128, CH], f32, tag="pfc")
                for kt in range(KH):
                    nc.tensor.matmul(
                        ps[:, 0:B],
                        lhsT=wfc_sb[:, kt, vt * 128:(vt + 1) * 128],
                        rhs=ctx_sb[:, kt, :],
                        start=(kt == 0),
                        stop=(kt == KH - 1),
                    )
                nc.scalar.activation(
                    ctxl_sb[:, vt], ps[:, 0:B], AF.Identity,
                    bias=bfc_sb[:, vt:vt + 1],
                )
            # ctxlT[b, v] (bf16) for the selection matmuls
            for vt in range(VT):
                ps = pfc.tile([128, CH], f32, tag="pfc")
                for kt in range(KH):
                    nc.tensor.matmul(
                        ps[0:B, 0:128],
                        lhsT=ctx_sb[:, kt, :],
                        rhs=wfc_sb[:, kt, vt * 128:(vt + 1) * 128],
                        start=(kt == 0),
                        stop=(kt == KH - 1),
                    )
                nc.scalar.activation(ctxlT_sb[0:B, vt], ps[0:B, 0:128], AF.Copy)

            # ---- FC: logits = hs @ wfh.T (+ctx +bias), vocab-sharded -
            # conversion unit = 512 tokens (two DoubleRow 256-groups in
            # one PSUM bank) to amortize the fixed PSUM access latency
            selv_of = {16: [0, 1, 2, 3], 8: [4, 5]}
            nconv = 0
            for j in range(NBLK):
                W = BLOCKS[j] * B          # tau width of this block
                for hb in range(W // 512):
                    for q in range(8):     # vt quads
                        fo = foutp.tile([128, 4, 512], bf, tag="fo")
                        for i in range(4):
                            vt = q * 4 + i
                            eng = CONV_PAT[nconv % len(CONV_PAT)]
                            nconv += 1
                            ps = pfc.tile([128, 512], f32, tag="pfc")
                            for cm in range(2):
                                m = hb * 2 + cm
                                n0 = TAUB[j] + m * CH
                                for kp in range(2):
                                    nc.tensor.matmul(
                                        ps[:, cm * CH:(cm + 1) * CH],
                                        lhsT=wfh_sb[:, 2 * kp:2 * kp + 2,
                                                    vt * 128:(vt + 1) * 128],
                                        rhs=hs_sb[:, 2 * kp:2 * kp + 2,
                                                  n0:n0 + CH],
                                        start=(kp == 0),
                                        stop=(eng == "D" and kp == 1),
                                        perf_mode=DR,
                                    )
                                if eng != "D":
                                    # ctx+bias via selection matmul
                                    sv = selv_of[BLOCKS[j]][m]
                                    nc.tensor.matmul(
                                        ps[:, cm * CH:(cm + 1) * CH],
                                        lhsT=ctxlT_sb[0:65, vt, :],
                                        rhs=sel_sb[0:65, sv, :],
                                        start=False,
                                        stop=True,
                                    )
                            out_sl = fo[:, i, :]
                            if eng == "D":
                                # ctx+bias via broadcast add on DVE
                                cpc = NCORES * 512 // W
                                ctxv = (
                                    ctxl_sb[:, vt,
                                            hb * cpc * BL:(hb + 1) * cpc * BL]
                                    .rearrange("p (c b) -> p c b", b=BL)
                                    .unsqueeze(2)
                                    .broadcast_to([128, cpc, BLOCKS[j], BL])
                                )
                                nc.vector.tensor_add(
                                    out_sl.rearrange(
                                        "p (c t b) -> p c t b",
                                        c=cpc, t=BLOCKS[j], b=BL),
                                    ps[:].rearrange(
                                        "p (c t b) -> p c t b",
                                        c=cpc, t=BLOCKS[j], b=BL),
                                    ctxv,
                                )
                            else:
                                nc.scalar.activation(out_sl, ps[:], AF.Copy)
                        nc.sync.dma_start(
                            out=log_d[q * 512:(q + 1) * 512,
                                      TAUB[j] + hb * 512:
                                      TAUB[j] + (hb + 1) * 512]
                            .rearrange("(i p) n -> p i n", p=128),
                            in_=fo[:],
                        )

    nc.compile()
    return nc


def _get_nc():
    if "nc" not in _CACHE:
        _CACHE["nc"] = _build_program()
    return _CACHE["nc"]


def _block128(a):
    """[K, N] -> [K//128, 128, N] contiguous blocks."""
    k, n = a.shape
    return np.ascontiguousarray(a.reshape(k // 128, 128, n))


def _t_layout(a):
    """[BL, 512] state -> [128, KH, BL] transposed tile layout."""
    return np.ascontiguousarray(a.T.reshape(KH, 128, BL).transpose(1, 0, 2))


def _build_sel():
    """Selection matrices [65, NSEL, 256] fp8.

    Variant layout (chunk of 256 gathered tokens, pos = (c_rel, t, b)):
      16-step blocks, chunk m in 0..3: batch = 16m + 8*(pos>>7) + (pos&7)
      8-step blocks, chunk m in 0..1:  batch = 32m + 8*(pos>>6) + (pos&7)
      4-step blocks, single chunk:     batch = 8*(pos>>5) + (pos&7)
    Row 64 is the all-ones bias row.
    """
    sel = np.zeros((65, NSEL, CH), np.float32)
    pos = np.arange(CH)
    for m in range(4):
        bt = 16 * m + 8 * (pos >> 7) + (pos & 7)
        sel[bt, m, pos] = 1.0
    for m in range(2):
        bt = 32 * m + 8 * (pos >> 6) + (pos & 7)
        sel[bt, 4 + m, pos] = 1.0
    bt = 8 * (pos >> 5) + (pos & 7)
    sel[bt, 6, pos] = 1.0
    sel[64, :, :] = 1.0
    return sel.astype(FP8)


def _prep_in_maps(target_seq, context, h, c, emb_table, w_ih, w_hh, b_ih,
                  b_hh, w_fc, b_fc):
    target_seq = np.asarray(target_seq)
    context = np.asarray(context, dtype=np.float32)
    h = np.asarray(h, dtype=np.float32)
    c = np.asarray(c, dtype=np.float32)
    emb_table = np.asarray(emb_table, dtype=np.float32)
    w_ih = np.asarray(w_ih, dtype=np.float32)
    w_hh = np.asarray(w_hh, dtype=np.float32)
    b_ih = np.asarray(b_ih, dtype=np.float32)
    b_hh = np.asarray(b_hh, dtype=np.float32)
    w_fc = np.asarray(w_fc, dtype=np.float32)
    b_fc = np.asarray(b_fc, dtype=np.float32)

    # gate-block permutation [i, f, g, o] -> [i, f, o, g]
    perm = np.concatenate([
        np.arange(0, HIDDEN),                    # i
        np.arange(HIDDEN, 2 * HIDDEN),           # f
        np.arange(3 * HIDDEN, 4 * HIDDEN),       # o
        np.arange(2 * HIDDEN, 3 * HIDDEN),       # g
    ])
    w_ih_p = w_ih[perm]
    w_hh_p = w_hh[perm]
    bias_p = (b_ih + b_hh)[perm]

    # w_ih with the bias as a 7th k-tile (input row is constant 1)
    wih_full = np.zeros((KI * 128, G4), np.float32)
    wih_full[:768] = w_ih_p.T
    wih_full[768] = bias_p
    wih_d = _block128(wih_full.astype(BF16))           # [7,128,2048]
    whh_d = _block128(w_hh_p.T.astype(FP8))            # [4,128,2048]
    ctx_d = _block128(context.T.astype(BF16))          # [4,128,64]
    sel_dm = np.ascontiguousarray(
        _build_sel())                                  # [65,NSEL,256]
    bfcT_d = np.ascontiguousarray(
        b_fc[:VSH].reshape(1, VT, 128)).astype(BF16)   # placeholder per-core

    emb = emb_table[target_seq]                        # [B,S,E] f32

    in_maps = []
    for cid in range(NCORES):
        bs = slice(cid * BL, (cid + 1) * BL)
        # lstm_in transposed, local tokens n = t*8+b; 897th row = 1
        x_loc = np.zeros((TOKL, KI * 128), np.float32)
        x_loc[:, :EMBED] = emb[bs].transpose(1, 0, 2).reshape(TOKL, EMBED)
        x_loc[:, EMBED:768] = np.tile(context[bs], (S, 1))
        x_loc[:, 768] = 1.0
        x_d = _block128(x_loc.T.astype(BF16))          # [7,128,512]
        vs = V_STARTS[cid]
        wfh_d = _block128(
            np.ascontiguousarray(w_fc[vs:vs + VSH, :HIDDEN].T).astype(FP8))
        wfc_d = _block128(
            np.ascontiguousarray(w_fc[vs:vs + VSH, HIDDEN:].T).astype(BF16))
        bfc_d = np.ascontiguousarray(
            b_fc[vs:vs + VSH].reshape(VT, 128).T.astype(np.float32))
        bfcT_c = np.ascontiguousarray(
            b_fc[vs:vs + VSH].reshape(1, VT, 128)).astype(BF16)
        in_maps.append({
            "x_d": x_d,
            "wih_d": wih_d,
            "whh_d": whh_d,
            "h0_d": _t_layout(h[bs]).astype(BF16),
            "c0_d": _t_layout(c[bs]).astype(np.float32),
            "ctx_d": ctx_d,
            "wfh_d": wfh_d,
            "wfc_d": wfc_d,
            "bfc_d": bfc_d,
            "bfcT_d": bfcT_c,
            "sel_d": sel_dm,
        })
    return in_maps


def _assemble(results):
    """results: list of per-core {"log_d": [4096, 4096] bf16} -> [B, S, V]."""
    full = np.empty((VOCAB, TOK), dtype=np.float32)
    for cid in range(NCORES):
        out_c = np.asarray(results[cid]["log_d"], dtype=np.float32)
        vs = V_STARTS[cid]
        r0 = cid * 4000 - vs
        full[cid * 4000:(cid + 1) * 4000] = out_c[r0:r0 + 4000]

    # tau = TAUB[j] + core*(BLOCKS[j]*8) + t_in*8 + b
    logits = np.empty((B, S, VOCAB), dtype=np.float32)
    for j in range(NBLK):
        W = BLOCKS[j] * B
        blk = full[:, TAUB[j]:TAUB[j] + W]             # [V, W]
        blk = blk.reshape(VOCAB, NCORES, BLOCKS[j], BL)
        logits[:, BSTART[j]:BSTART[j] + BLOCKS[j], :] = (
            blk.transpose(1, 3, 2, 0).reshape(B, BLOCKS[j], VOCAB)
        )
    return np.ascontiguousarray(logits)


def kernel(target_seq, context, h, c, emb_table, w_ih, w_hh, b_ih, b_hh,
           w_fc, b_fc):
    from concourse.bass_utils import run_bass_kernel_spmd

    in_maps = _prep_in_maps(target_seq, context, h, c, emb_table, w_ih,
                            w_hh, b_ih, b_hh, w_fc, b_fc)
    nc = _get_nc()
    res = run_bass_kernel_spmd(nc, in_maps, core_ids=list(range(NCORES)))
    return _assemble(res.results)
